# revision 47
# baseline (speedup 1.0000x reference)
"""Trainium2 Bass kernel for the SE-gated Non-local block (rank-1 attention).

Math (per batch item b, x viewed as [C, N] with N = H*W):
    S[c]    = sum_n x[c, n]                      (spatial sum)
    hid     = relu((se_w1 / N) @ S + se_b1)      (SE bottleneck; 1/N folds the mean)
    gate    = sigmoid(se_w2 @ hid + se_b2)       [C]
    w5e     = gate * [theta_w | 0 | 0 | g_w | phi_w]  [C, 5]
    prow    = w5e.T @ x + [th_b, 1, 1, g_b, phi_b]    [5, N]
              rows: theta, ONES, ONES, g, phi   (the ones rows come from the bias)
    s_raw   = sum_n prow[3] * prow[4]
    out     = x + As (outer) theta + (Bc_hi + Bc_lo) (outer) ones   where
              inv = bn_gamma / sqrt(bn_var + eps)
              As  = (W_w * inv / N) * s_raw      (1/N folds the f/N normalizer)
              Bc  = (W_b - bn_mean) * inv + bn_beta,  split hi/lo so the pair of
              bf16 rows carries ~f32 precision through the PE.

Schedule: HBM traffic is the floor (read x + write out = 37.7 MB/core).  Both
batch items' x stay f32-resident in SBUF (2 x 72 KB/partition) so item 1's
loads overlap item 0's compute/stores; loads alternate the two HWDGE rings so
consecutive-DMA bubbles overlap.  No bf16 copy pass on DVE/GpSimd (both are
~5-17x slower than ACT at f32->bf16): the fused ACT identity produces the
bf16 x chunks AND the spatial sums in one line-rate pass.  The rank-2
correction As*theta + Bc*1 is computed by the TensorEngine as a 3-row bf16
matmul (PSUM), so the output pass is one DVE add per block.  Engine split:
ACT = cast+rowsum + SE, DVE = psum copies + adds + dots, GpSimd = weights +
g/phi reshape DMAs (SWDGE), PE = all matmuls, sync ring = even loads +
stores, scalar ring = odd loads.

The g.phi dot needs n on partitions: SBUF->SBUF reshape DMAs move each
produced third of the g/phi rows [1, 1536] -> [128, 12] (same n-permutation
for both rows, so the dot is unchanged), with the partial dot folded in as
each group lands.
"""

import numpy as np

B, C, H, W = 16, 512, 96, 48
N = H * W            # 4608
P = 128
KC = C // P          # 4 channel chunks
NB = 512             # free-dim block = one fp32 PSUM bank
NJ = N // NB         # 9
NCORES = 8
BPC = B // NCORES    # 2 batch items per core
SE_C = C // 16       # 32
MR = N // P          # 36: elems per partition in the reshaped g/phi rows
BN_EPS = 1e-5

_CACHE = {}
LAST_RESULTS = None


def _build_bass(stage="full"):
    # stage: bisection aid — "loads", "se", "proj", "rows", "full"
    S = {"loads": 0, "se": 1, "proj": 2, "rows": 3, "full": 4}[stage]
    import concourse.mybir as mybir
    from concourse.bacc import Bacc
    from concourse.tile import TileContext

    f32 = mybir.dt.float32
    bf16 = mybir.dt.bfloat16
    AF = mybir.ActivationFunctionType
    AX = mybir.AxisListType

    nc = Bacc(num_swdge_queues=2)
    xs = nc.dram_tensor("xs", [BPC, C, N], f32, kind="ExternalInput")
    w1 = nc.dram_tensor("w1", [P, KC, SE_C], f32, kind="ExternalInput")
    w2 = nc.dram_tensor("w2", [SE_C, C], f32, kind="ExternalInput")
    b1 = nc.dram_tensor("b1", [SE_C, 1], f32, kind="ExternalInput")
    b2 = nc.dram_tensor("b2", [P, KC], f32, kind="ExternalInput")
    w5 = nc.dram_tensor("w5", [P, KC, 5], f32, kind="ExternalInput")
    pb = nc.dram_tensor("pb", [5, 1], f32, kind="ExternalInput")
    ar = nc.dram_tensor("ar", [1, C], f32, kind="ExternalInput")    # W_w*inv/N
    bchl = nc.dram_tensor("bchl", [2, C], bf16, kind="ExternalInput")  # Bc hi/lo
    out_d = nc.dram_tensor("out", [BPC, C, N], f32, kind="ExternalOutput")

    with TileContext(nc) as tc:
        with (
            tc.tile_pool(name="wpool", bufs=1) as wpool,
            tc.tile_pool(name="xpool", bufs=2 * KC) as xpool,
            tc.tile_pool(name="bpool", bufs=KC) as bpool,
            tc.tile_pool(name="ppool", bufs=1) as ppool,
            tc.tile_pool(name="spool", bufs=2) as spool,
            tc.tile_pool(name="ps_se", bufs=2, space="PSUM") as ps_se,
            tc.tile_pool(name="ps_pj", bufs=2, space="PSUM") as ps_pj,
            tc.tile_pool(name="ps_cr", bufs=4, space="PSUM") as ps_cr,
        ):
            w1t = wpool.tile([P, KC, SE_C], f32, tag="w1t")
            w2t = wpool.tile([SE_C, C], f32, tag="w2t")
            b1t = wpool.tile([SE_C, 1], f32, tag="b1t")
            b2t = wpool.tile([P, KC], f32, tag="b2t")
            w5t = wpool.tile([P, KC, 5], f32, tag="w5t")
            pbt = wpool.tile([5, 1], f32, tag="pbt")
            art = wpool.tile([1, C], f32, tag="art")
            ab3 = wpool.tile([3, C], bf16, tag="ab3")     # rows: As, Bc_hi, Bc_lo
            on128 = wpool.tile([P, P], f32, tag="on128")  # all-ones (part. sum)

            nc.vector.memset(on128[:], 1.0)
            for t, d in ((w1t, w1), (w2t, w2), (b1t, b1), (b2t, b2),
                         (w5t, w5), (pbt, pb), (art, ar)):
                nc.gpsimd.dma_start(out=t[:], in_=d[:])
            nc.gpsimd.dma_start(out=ab3[1:3, :], in_=bchl[:])

            # preload the ACT sigmoid table while idle (else the first SE
            # sigmoid pays a ~1.3us ACT_TABLE_LOAD on the critical path)
            dmy = spool.tile([1, 1], f32, tag="dmy")
            nc.vector.memset(dmy[:], 0.0)
            nc.scalar.activation(out=dmy[:], in_=dmy[:], func=AF.Sigmoid)

            # ---- load x (f32, both items resident); alternate the two
            #      HWDGE rings so consecutive-DMA bubbles overlap ----
            xts = [[None] * KC for _ in range(BPC)]
            for b in range(BPC):
                for k in range(KC):
                    xt = xpool.tile([P, N], f32, tag="xt")
                    eng = nc.sync if (b * KC + k) % 2 == 0 else nc.scalar
                    eng.dma_start(out=xt[:], in_=xs[b, k * P:(k + 1) * P, :])
                    xts[b][k] = xt

            for b in range(BPC):
                # ---- fused bf16 cast + spatial sums on ACT (one pass) ----
                xp = spool.tile([P, KC], f32, tag="xp")
                xbs = []
                for k in range(KC):
                    xb = bpool.tile([P, N], bf16, tag="xb")
                    nc.scalar.activation(out=xb[:], in_=xts[b][k][:],
                                         func=AF.Identity,
                                         accum_out=xp[:, k:k + 1])
                    xbs.append(xb)

                if S >= 1:
                    # ---- SE gate ----
                    php = ps_se.tile([SE_C, 1], f32, tag="ps_se")
                    for k in range(KC):
                        nc.tensor.matmul(php[:], w1t[:, k, :], xp[:, k:k + 1],
                                         start=(k == 0), stop=(k == KC - 1))
                    hid = spool.tile([SE_C, 1], f32, tag="hid")
                    nc.scalar.activation(out=hid[:], in_=php[:], func=AF.Relu,
                                         bias=b1t[:], scale=1.0)
                    gate = spool.tile([P, KC], f32, tag="gate")
                    for k in range(KC):
                        gp = ps_se.tile([P, 1], f32, tag="ps_se")
                        nc.tensor.matmul(gp[:], w2t[:, k * P:(k + 1) * P],
                                         hid[:], start=True, stop=True)
                        nc.scalar.activation(out=gate[:, k:k + 1], in_=gp[:],
                                             func=AF.Sigmoid,
                                             bias=b2t[:, k:k + 1], scale=1.0)

                if S >= 2:
                    # ---- gated projections: prow = w5e.T @ x (bf16 PE) ----
                    w5e = spool.tile([P, KC, 5], bf16, tag="w5e")
                    for k in range(KC):
                        nc.vector.tensor_scalar_mul(out=w5e[:, k, :],
                                                    in0=w5t[:, k, :],
                                                    scalar1=gate[:, k:k + 1])
                    prow = ppool.tile([5, N], bf16, tag="prow")
                    g_rs = spool.tile([P, MR], bf16, tag="g_rs")
                    p_rs = spool.tile([P, MR], bf16, tag="p_rs")
                    GW = 3 * NB   # reshape-group width in n (1536)
                    M3 = MR // 3  # 12 reshaped columns per group
                    for j in range(NJ):
                        pp = ps_pj.tile([5, NB], f32, tag="pp")
                        for k in range(KC):
                            nc.tensor.matmul(pp[:], w5e[:, k, :],
                                             xbs[k][:, j * NB:(j + 1) * NB],
                                             start=(k == 0), stop=(k == KC - 1))
                        nc.vector.tensor_scalar_add(
                            out=prow[:, j * NB:(j + 1) * NB],
                            in0=pp[:], scalar1=pbt[:])
                        if S >= 3 and j in (3, 7, 8):
                            # stream g/phi rows into [128, .] layout as they
                            # are produced (SWDGE; same n-permutation for
                            # both rows so the dot is unchanged).  Groups of
                            # 4/4/1 blocks: the LAST reshape pair on the
                            # s_raw critical path is a tiny 512-col transfer
                            lo = {3: 0, 7: 4, 8: 8}[j]
                            nsl = slice(lo * NB, (j + 1) * NB)
                            msl = slice(lo * (NB // P), (j + 1) * (NB // P))
                            nc.gpsimd.dma_start(out=g_rs[:, msl],
                                                in_=prow[3:4, nsl])
                            nc.gpsimd.dma_start(out=p_rs[:, msl],
                                                in_=prow[4:5, nsl])

                if S >= 3:
                    # ---- s_raw = <g, phi>; single dot AFTER all prow copies
                    #      (a per-group dot interleaves DMA-waits into the
                    #      DVE FIFO and stalls the later psum copies) ----
                    prod = spool.tile([P, MR], f32, tag="prod")
                    nc.vector.tensor_mul(out=prod[:], in0=g_rs[:], in1=p_rs[:])
                    r1 = spool.tile([P, 1], f32, tag="r1")
                    nc.vector.reduce_sum(out=r1[:], in_=prod[:], axis=AX.X)
                    sb = ps_se.tile([P, 1], f32, tag="ps_se")
                    nc.tensor.matmul(sb[:], on128[:], r1[:], start=True,
                                     stop=True)
                    # As row = (W_w*inv/N) * s_raw, into ab3 row 0 (bf16)
                    nc.vector.tensor_scalar_mul(out=ab3[0:1, :], in0=art[:],
                                                scalar1=sb[0:1, 0:1])

                # ---- corr = As x theta + Bc x ones via PE (3-row bf16);
                #      out = x + corr in place; store per chunk ----
                for k in range(KC):
                    if S >= 4:
                        for j in range(NJ):
                            cp = ps_cr.tile([P, NB], f32, tag="cp")
                            nc.tensor.matmul(
                                cp[:], ab3[:, k * P:(k + 1) * P],
                                prow[0:3, j * NB:(j + 1) * NB],
                                start=True, stop=True)
                            sl = slice(j * NB, (j + 1) * NB)
                            nc.vector.tensor_add(out=xts[b][k][:, sl],
                                                 in0=xts[b][k][:, sl],
                                                 in1=cp[:])
                    nc.sync.dma_start(out=out_d[b, k * P:(k + 1) * P, :],
                                      in_=xts[b][k][:])

    nc.finalize()
    return nc


def kernel(**inputs):
    global LAST_RESULTS
    from concourse.bass_utils import run_bass_kernel_spmd
    import ml_dtypes

    a = {k: np.asarray(v, dtype=np.float32) for k, v in inputs.items()}
    x = np.ascontiguousarray(a["x"]).reshape(B, C, N)

    inv = a["bn_gamma"] / np.sqrt(a["bn_var"] + BN_EPS)
    A = (a["W_w"] * inv / N).astype(np.float32)
    Bc = ((a["W_b"] - a["bn_mean"]) * inv + a["bn_beta"]).astype(np.float32)
    Bc_hi = Bc.astype(ml_dtypes.bfloat16)
    Bc_lo = (Bc - Bc_hi.astype(np.float32)).astype(ml_dtypes.bfloat16)

    w1h = np.ascontiguousarray(
        (a["se_w1"] / N).T.reshape(KC, P, SE_C).transpose(1, 0, 2)).astype(np.float32)
    w2h = np.ascontiguousarray(a["se_w2"].T).astype(np.float32)
    b1h = np.ascontiguousarray(a["se_b1"].reshape(SE_C, 1))
    b2h = np.ascontiguousarray(a["se_b2"].reshape(KC, P).T)
    zc = np.zeros(C, np.float32)
    w5h = np.ascontiguousarray(
        np.stack([a["theta_w"], zc, zc, a["g_w"], a["phi_w"]],
                 axis=1).reshape(KC, P, 5).transpose(1, 0, 2)).astype(np.float32)
    pbh = np.array([[a["theta_b"]], [1.0], [1.0], [a["g_b"]], [a["phi_b"]]],
                   dtype=np.float32)
    arh = np.ascontiguousarray(A.reshape(1, C))
    bchlh = np.ascontiguousarray(np.stack([Bc_hi, Bc_lo], axis=0))

    if "nc" not in _CACHE:
        _CACHE["nc"] = _build_bass()
    nc = _CACHE["nc"]

    in_maps = []
    for c in range(NCORES):
        in_maps.append({
            "xs": np.ascontiguousarray(x[c * BPC:(c + 1) * BPC]),
            "w1": w1h, "w2": w2h, "b1": b1h, "b2": b2h,
            "w5": w5h, "pb": pbh, "ar": arh, "bchl": bchlh,
        })

    res = run_bass_kernel_spmd(nc, in_maps, core_ids=list(range(NCORES)))
    LAST_RESULTS = res

    out = np.concatenate([res.results[c]["out"] for c in range(NCORES)], axis=0)
    return np.ascontiguousarray(out.reshape(B, C, H, W))


# revision 49
# speedup vs baseline: 1.0187x; 1.0187x over previous
"""Trainium2 Bass kernel for the SE-gated Non-local block (rank-1 attention).

Math (per batch item b, x viewed as [C, N] with N = H*W):
    S[c]    = sum_n x[c, n]                      (spatial sum)
    hid     = relu((se_w1 / N) @ S + se_b1)      (SE bottleneck; 1/N folds the mean)
    gate    = sigmoid(se_w2 @ hid + se_b2)       [C]
    w5e     = gate * [theta_w | 0 | 0 | g_w | phi_w]  [C, 5]
    prow    = w5e.T @ x + [th_b, 1, 1, g_b, phi_b]    [5, N]
              rows: theta, ONES, ONES, g, phi   (the ones rows come from the bias)
    s_raw   = sum_n prow[3] * prow[4]
    out     = x + As (outer) theta + (Bc_hi + Bc_lo) (outer) ones   where
              inv = bn_gamma / sqrt(bn_var + eps)
              As  = (W_w * inv / N) * s_raw      (1/N folds the f/N normalizer)
              Bc  = (W_b - bn_mean) * inv + bn_beta,  split hi/lo so the pair of
              bf16 rows carries ~f32 precision through the PE.

Schedule: HBM traffic is the floor (read x + write out = 37.7 MB/core).  Both
batch items' x stay f32-resident in SBUF (2 x 72 KB/partition) so item 1's
loads overlap item 0's compute/stores; loads alternate the two HWDGE rings so
consecutive-DMA bubbles overlap.  No bf16 copy pass on DVE/GpSimd (both are
~5-17x slower than ACT at f32->bf16): the fused ACT identity produces the
bf16 x chunks AND the spatial sums in one line-rate pass.  The rank-2
correction As*theta + Bc*1 is computed by the TensorEngine as a 3-row bf16
matmul (PSUM), so the output pass is one DVE add per block.  Engine split:
ACT = cast+rowsum + SE, DVE = psum copies + adds + dots, GpSimd = weights +
g/phi reshape DMAs (SWDGE), PE = all matmuls, sync ring = even loads +
stores, scalar ring = odd loads.

The g.phi dot needs n on partitions: SBUF->SBUF reshape DMAs move each
produced third of the g/phi rows [1, 1536] -> [128, 12] (same n-permutation
for both rows, so the dot is unchanged), with the partial dot folded in as
each group lands.
"""

import numpy as np

B, C, H, W = 16, 512, 96, 48
N = H * W            # 4608
P = 128
KC = C // P          # 4 channel chunks
NB = 512             # free-dim block = one fp32 PSUM bank
NJ = N // NB         # 9
NCORES = 8
BPC = B // NCORES    # 2 batch items per core
SE_C = C // 16       # 32
MR = N // P          # 36: elems per partition in the reshaped g/phi rows
BN_EPS = 1e-5

_CACHE = {}
LAST_RESULTS = None


def _build_bass(stage="full"):
    # stage: bisection aid — "loads", "se", "proj", "rows", "full"
    S = {"loads": 0, "se": 1, "proj": 2, "rows": 3, "full": 4}[stage]
    import concourse.mybir as mybir
    from concourse.bacc import Bacc
    from concourse.tile import TileContext

    f32 = mybir.dt.float32
    bf16 = mybir.dt.bfloat16
    AF = mybir.ActivationFunctionType
    AX = mybir.AxisListType

    nc = Bacc()
    xs = nc.dram_tensor("xs", [BPC, C, N], f32, kind="ExternalInput")
    w1 = nc.dram_tensor("w1", [P, KC, SE_C], f32, kind="ExternalInput")
    w2 = nc.dram_tensor("w2", [SE_C, C], f32, kind="ExternalInput")
    b1 = nc.dram_tensor("b1", [SE_C, 1], f32, kind="ExternalInput")
    b2 = nc.dram_tensor("b2", [P, KC], f32, kind="ExternalInput")
    w5 = nc.dram_tensor("w5", [P, KC, 5], f32, kind="ExternalInput")
    pb = nc.dram_tensor("pb", [5, 1], f32, kind="ExternalInput")
    ar = nc.dram_tensor("ar", [1, C], f32, kind="ExternalInput")    # W_w*inv/N
    bchl = nc.dram_tensor("bchl", [2, C], bf16, kind="ExternalInput")  # Bc hi/lo
    out_d = nc.dram_tensor("out", [BPC, C, N], f32, kind="ExternalOutput")

    with TileContext(nc) as tc:
        with (
            tc.tile_pool(name="wpool", bufs=1) as wpool,
            tc.tile_pool(name="xpool", bufs=2 * KC) as xpool,
            tc.tile_pool(name="bpool", bufs=KC) as bpool,
            tc.tile_pool(name="ppool", bufs=1) as ppool,
            tc.tile_pool(name="spool", bufs=2) as spool,
            tc.tile_pool(name="ps_se", bufs=2, space="PSUM") as ps_se,
            tc.tile_pool(name="ps_pj", bufs=2, space="PSUM") as ps_pj,
            tc.tile_pool(name="ps_cr", bufs=4, space="PSUM") as ps_cr,
        ):
            w1t = wpool.tile([P, KC, SE_C], f32, tag="w1t")
            w2t = wpool.tile([SE_C, C], f32, tag="w2t")
            b1t = wpool.tile([SE_C, 1], f32, tag="b1t")
            b2t = wpool.tile([P, KC], f32, tag="b2t")
            w5t = wpool.tile([P, KC, 5], f32, tag="w5t")
            pbt = wpool.tile([5, 1], f32, tag="pbt")
            art = wpool.tile([1, C], f32, tag="art")
            ab3 = wpool.tile([3, C], bf16, tag="ab3")     # rows: As, Bc_hi, Bc_lo
            on128 = wpool.tile([P, P], f32, tag="on128")  # all-ones (part. sum)

            nc.vector.memset(on128[:], 1.0)
            for t, d in ((w1t, w1), (w2t, w2), (b1t, b1), (b2t, b2),
                         (w5t, w5), (pbt, pb), (art, ar)):
                nc.gpsimd.dma_start(out=t[:], in_=d[:])
            nc.gpsimd.dma_start(out=ab3[1:3, :], in_=bchl[:])

            # preload the ACT sigmoid table while idle (else the first SE
            # sigmoid pays a ~1.3us ACT_TABLE_LOAD on the critical path)
            dmy = spool.tile([1, 1], f32, tag="dmy")
            nc.vector.memset(dmy[:], 0.0)
            nc.scalar.activation(out=dmy[:], in_=dmy[:], func=AF.Sigmoid)

            # ---- load x (f32, both items resident); alternate the two
            #      HWDGE rings so consecutive-DMA bubbles overlap ----
            xts = [[None] * KC for _ in range(BPC)]
            for b in range(BPC):
                for k in range(KC):
                    xt = xpool.tile([P, N], f32, tag="xt")
                    eng = nc.sync if (b * KC + k) % 2 == 0 else nc.scalar
                    eng.dma_start(out=xt[:], in_=xs[b, k * P:(k + 1) * P, :])
                    xts[b][k] = xt

            for b in range(BPC):
                # ---- fused bf16 cast + spatial sums on ACT (one pass) ----
                xp = spool.tile([P, KC], f32, tag="xp")
                xbs = []
                for k in range(KC):
                    xb = bpool.tile([P, N], bf16, tag="xb")
                    nc.scalar.activation(out=xb[:], in_=xts[b][k][:],
                                         func=AF.Identity,
                                         accum_out=xp[:, k:k + 1])
                    xbs.append(xb)

                if S >= 1:
                    # ---- SE gate ----
                    php = ps_se.tile([SE_C, 1], f32, tag="ps_se")
                    for k in range(KC):
                        nc.tensor.matmul(php[:], w1t[:, k, :], xp[:, k:k + 1],
                                         start=(k == 0), stop=(k == KC - 1))
                    hid = spool.tile([SE_C, 1], f32, tag="hid")
                    nc.scalar.activation(out=hid[:], in_=php[:], func=AF.Relu,
                                         bias=b1t[:], scale=1.0)
                    gate = spool.tile([P, KC], f32, tag="gate")
                    for k in range(KC):
                        gp = ps_se.tile([P, 1], f32, tag="ps_se")
                        nc.tensor.matmul(gp[:], w2t[:, k * P:(k + 1) * P],
                                         hid[:], start=True, stop=True)
                        nc.scalar.activation(out=gate[:, k:k + 1], in_=gp[:],
                                             func=AF.Sigmoid,
                                             bias=b2t[:, k:k + 1], scale=1.0)

                if S >= 2:
                    # ---- gated projections: prow = w5e.T @ x (bf16 PE) ----
                    w5e = spool.tile([P, KC, 5], bf16, tag="w5e")
                    for k in range(KC):
                        nc.vector.tensor_scalar_mul(out=w5e[:, k, :],
                                                    in0=w5t[:, k, :],
                                                    scalar1=gate[:, k:k + 1])
                    prow = ppool.tile([5, N], bf16, tag="prow")
                    g_rs = spool.tile([P, MR], bf16, tag="g_rs")
                    p_rs = spool.tile([P, MR], bf16, tag="p_rs")
                    GW = 3 * NB   # reshape-group width in n (1536)
                    M3 = MR // 3  # 12 reshaped columns per group
                    for j in range(NJ):
                        pp = ps_pj.tile([5, NB], f32, tag="pp")
                        for k in range(KC):
                            nc.tensor.matmul(pp[:], w5e[:, k, :],
                                             xbs[k][:, j * NB:(j + 1) * NB],
                                             start=(k == 0), stop=(k == KC - 1))
                        nc.vector.tensor_scalar_add(
                            out=prow[:, j * NB:(j + 1) * NB],
                            in0=pp[:], scalar1=pbt[:])
                        if S >= 3 and j in (3, 7, 8):
                            # stream g/phi rows into [128, .] layout as they
                            # are produced (SWDGE; same n-permutation for
                            # both rows so the dot is unchanged).  Groups of
                            # 4/4/1 blocks: the LAST reshape pair on the
                            # s_raw critical path is a tiny 512-col transfer
                            lo = {3: 0, 7: 4, 8: 8}[j]
                            nsl = slice(lo * NB, (j + 1) * NB)
                            msl = slice(lo * (NB // P), (j + 1) * (NB // P))
                            nc.scalar.dma_start(out=g_rs[:, msl],
                                                in_=prow[3:4, nsl])
                            nc.scalar.dma_start(out=p_rs[:, msl],
                                                in_=prow[4:5, nsl])

                if S >= 3:
                    # ---- s_raw = <g, phi>; single dot AFTER all prow copies
                    #      (a per-group dot interleaves DMA-waits into the
                    #      DVE FIFO and stalls the later psum copies) ----
                    prod = spool.tile([P, MR], f32, tag="prod")
                    nc.vector.tensor_mul(out=prod[:], in0=g_rs[:], in1=p_rs[:])
                    r1 = spool.tile([P, 1], f32, tag="r1")
                    nc.vector.reduce_sum(out=r1[:], in_=prod[:], axis=AX.X)
                    sb = ps_se.tile([P, 1], f32, tag="ps_se")
                    nc.tensor.matmul(sb[:], on128[:], r1[:], start=True,
                                     stop=True)
                    # As row = (W_w*inv/N) * s_raw, into ab3 row 0 (bf16)
                    nc.vector.tensor_scalar_mul(out=ab3[0:1, :], in0=art[:],
                                                scalar1=sb[0:1, 0:1])

                # ---- corr = As x theta + Bc x ones via PE (3-row bf16);
                #      out = x + corr in place; store per chunk ----
                for k in range(KC):
                    if S >= 4:
                        for j in range(NJ):
                            cp = ps_cr.tile([P, NB], f32, tag="cp")
                            nc.tensor.matmul(
                                cp[:], ab3[:, k * P:(k + 1) * P],
                                prow[0:3, j * NB:(j + 1) * NB],
                                start=True, stop=True)
                            sl = slice(j * NB, (j + 1) * NB)
                            nc.vector.tensor_add(out=xts[b][k][:, sl],
                                                 in0=xts[b][k][:, sl],
                                                 in1=cp[:])
                    nc.sync.dma_start(out=out_d[b, k * P:(k + 1) * P, :],
                                      in_=xts[b][k][:])

    nc.finalize()
    return nc


def kernel(**inputs):
    global LAST_RESULTS
    from concourse.bass_utils import run_bass_kernel_spmd
    import ml_dtypes

    a = {k: np.asarray(v, dtype=np.float32) for k, v in inputs.items()}
    x = np.ascontiguousarray(a["x"]).reshape(B, C, N)

    inv = a["bn_gamma"] / np.sqrt(a["bn_var"] + BN_EPS)
    A = (a["W_w"] * inv / N).astype(np.float32)
    Bc = ((a["W_b"] - a["bn_mean"]) * inv + a["bn_beta"]).astype(np.float32)
    Bc_hi = Bc.astype(ml_dtypes.bfloat16)
    Bc_lo = (Bc - Bc_hi.astype(np.float32)).astype(ml_dtypes.bfloat16)

    w1h = np.ascontiguousarray(
        (a["se_w1"] / N).T.reshape(KC, P, SE_C).transpose(1, 0, 2)).astype(np.float32)
    w2h = np.ascontiguousarray(a["se_w2"].T).astype(np.float32)
    b1h = np.ascontiguousarray(a["se_b1"].reshape(SE_C, 1))
    b2h = np.ascontiguousarray(a["se_b2"].reshape(KC, P).T)
    zc = np.zeros(C, np.float32)
    w5h = np.ascontiguousarray(
        np.stack([a["theta_w"], zc, zc, a["g_w"], a["phi_w"]],
                 axis=1).reshape(KC, P, 5).transpose(1, 0, 2)).astype(np.float32)
    pbh = np.array([[a["theta_b"]], [1.0], [1.0], [a["g_b"]], [a["phi_b"]]],
                   dtype=np.float32)
    arh = np.ascontiguousarray(A.reshape(1, C))
    bchlh = np.ascontiguousarray(np.stack([Bc_hi, Bc_lo], axis=0))

    if "nc" not in _CACHE:
        _CACHE["nc"] = _build_bass()
    nc = _CACHE["nc"]

    in_maps = []
    for c in range(NCORES):
        in_maps.append({
            "xs": np.ascontiguousarray(x[c * BPC:(c + 1) * BPC]),
            "w1": w1h, "w2": w2h, "b1": b1h, "b2": b2h,
            "w5": w5h, "pb": pbh, "ar": arh, "bchl": bchlh,
        })

    res = run_bass_kernel_spmd(nc, in_maps, core_ids=list(range(NCORES)))
    LAST_RESULTS = res

    out = np.concatenate([res.results[c]["out"] for c in range(NCORES)], axis=0)
    return np.ascontiguousarray(out.reshape(B, C, H, W))
